# revision 7
# baseline (speedup 1.0000x reference)
"""TRN2 Bass kernel for CompressedLinearLayer: out = x @ (A @ B.T).T + bias.

Computed low-rank: t = x @ B  (rank 512), out = t @ A.T + bias.
Sharding: data-parallel over the 8192 rows of x (1024 rows per core);
B, A.T, bias replicated. No collectives.

Host pre-tiles every input into the exact SBUF layout so each device
DMA is one contiguous 0.5MB transfer (4KB per-partition lines):
  x  [16, 128, 2048] bf16   (block*8+g, p, ks*512+m)
  b  [ 8, 128, 2048] bf16   (g, p, ks*512+r)
  at [ 8, 128, 2048] bf16   (dcg, p, k*512+d)
  bias [4096] f32
  out [1024, 4096] bf16     (host casts back to f32)

Ring split: x on sync HWDGE, b then at on scalar HWDGE, bias + most
output stores on the gpsimd ring -- descriptor issue runs in parallel
and b/x stream concurrently from t~7.5us.  A.T arrives as 8
just-in-time 0.5MB column-block tiles consumed dcg-major by stage 2.

Per core the 1024 rows are processed in 2 blocks of 512:
  stage1(b): tT[r, m] = sum_k B[k, r] * xT[k, m]   (rank on partitions)
  stage2(b): out[m, d] = sum_r tT[r, m] * AT[r, d] + bias[d]
Interleave [s1(1)g0, s2dcg0, s1(1)g1, s2dcg1, ...] keeps the PE fed
across the block transition while tT evacuations drain on the DVE.
Accumulation is fp32 in PSUM; bias is added during PSUM evacuation on
the vector engine; outputs store as bf16 (final tiles fine-grained
across rings to shrink the tail).
"""
import numpy as np
import ml_dtypes

import concourse.bacc as bacc
import concourse.mybir as mybir
import concourse.tile as tile
from concourse.bass_utils import run_bass_kernel_spmd

N_CORES = 8
BATCH, SEQ = 4, 2048
D_IN, D_OUT, RANK = 4096, 4096, 512
ROWS_TOTAL = BATCH * SEQ           # 8192
ROWS = ROWS_TOTAL // N_CORES       # 1024 rows per core

F32 = mybir.dt.float32
BF16 = mybir.dt.bfloat16

KC = D_IN // 128     # 32 contraction chunks, stage 1
KSUB = 4             # k-chunks packed per DMA (0.5MB bf16 transfers)
KB = KC // KSUB      # 8 packed k-groups
RC = RANK // 128     # 4 rank chunks
NBLK = 2             # row blocks per core
BROWS = ROWS // NBLK                 # 512 rows per block
MB2 = BROWS // 128   # 4 row chunks of 128 per block (stage-2 out partitions)
DCG = D_OUT // 512   # 8 d_out column groups of 512 (stage-2 at tiles)

_compiled = {}


def _build():
    nc = bacc.Bacc("TRN2", target_bir_lowering=False, debug=False)

    x_d = nc.declare_dram_parameter("x", [NBLK * KB * 128, KSUB * BROWS], BF16,
                                    isOutput=False)
    b_d = nc.declare_dram_parameter("b", [KB * 128, KSUB * RANK], BF16,
                                    isOutput=False)
    at_d = nc.declare_dram_parameter("at", [DCG * 128, RC * 512], BF16,
                                     isOutput=False)
    bias_d = nc.declare_dram_parameter("bias", [D_OUT], F32, isOutput=False)
    out_d = nc.declare_dram_parameter("out", [ROWS, D_OUT], BF16, isOutput=True)

    with tile.TileContext(nc) as tc:
        with (
            tc.tile_pool(name="wb", bufs=1) as wb,
            tc.tile_pool(name="op", bufs=4) as op,
            tc.tile_pool(name="ps1", bufs=4, space="PSUM") as ps1p,
            tc.tile_pool(name="ps2", bufs=4, space="PSUM") as ps2p,
        ):
            bias_bc = wb.tile([128, D_OUT], F32, tag="bias_bc")

            # B resident: 8 tiles [128, 2048] bf16 (ks*512+r)
            b_sb = [
                wb.tile([128, KSUB * RANK], BF16, tag=f"b{g}", name=f"b{g}")
                for g in range(KB)
            ]
            # A.T resident: 8 column-group tiles [128, 2048] bf16 (k*512+d)
            at_sb = [
                wb.tile([128, RC * 512], BF16, tag=f"at{c}", name=f"at{c}")
                for c in range(DCG)
            ]
            # x: every group gets its own buffer (x0 + x1 fully resident)
            x_sb = [
                wb.tile([128, KSUB * BROWS], BF16, tag=f"x{i}", name=f"x{i}")
                for i in range(NBLK * KB)
            ]
            # tT per block: 4 tiles [128, 512] bf16 each
            tT = [
                [
                    wb.tile([128, BROWS], BF16, tag=f"tT{b}_{r}",
                            name=f"tT{b}_{r}")
                    for r in range(RC)
                ]
                for b in range(NBLK)
            ]

            # ---- critical first chunks (b g0ks0 + x g0ks0) on the sync
            # ring, which cold-starts ~1.5us earlier than scalar ----
            nc.sync.dma_start(b_sb[0][:, 0:512], b_d[0:128, 0:512])
            nc.sync.dma_start(x_sb[0][:, 0:512], x_d[0:128, 0:512])
            nc.sync.dma_start(x_sb[0][:, 512:2048], x_d[0:128, 512:2048])
            # remaining b on scalar
            nc.scalar.dma_start(b_sb[0][:, 512:2048], b_d[0:128, 512:2048])
            for g in range(1, KB):
                nc.scalar.dma_start(b_sb[g][:], b_d[g * 128:(g + 1) * 128, :])
            # x split across sync (even groups) and gpsimd (odd groups)
            for gi in range(1, NBLK * KB):
                ring = nc.sync if gi % 2 == 0 else nc.gpsimd
                ring.dma_start(x_sb[gi][:], x_d[gi * 128:(gi + 1) * 128, :])
            # bias: behind the x issues so the broadcast compute does not
            # stall the gpsimd ring's descriptor stream
            nc.gpsimd.dma_start(bias_bc[0:1, :], bias_d[None, :])
            nc.gpsimd.partition_broadcast(bias_bc[:], bias_bc[0:1, :])

            # ---- at on scalar ring, queued behind b ----
            for c in range(DCG):
                nc.scalar.dma_start(at_sb[c][:], at_d[c * 128:(c + 1) * 128, :])

            def stage1_group(b, g, psum1):
                xg = x_sb[b * KB + g]
                last = g == KB - 1
                if not last:
                    for ks in range(KSUB):
                        k = g * KSUB + ks
                        for mc in range(RC):
                            nc.tensor.matmul(
                                psum1[mc][:],
                                b_sb[g][:, ks * 512 + mc * 128:
                                        ks * 512 + (mc + 1) * 128],
                                xg[:, ks * 512:(ks + 1) * 512],
                                start=(k == 0),
                                stop=False,
                            )
                else:
                    # invert loops so each psum finishes (and can evacuate to
                    # tT on the DVE) while the PE continues with the next mc
                    for mc in range(RC):
                        for ks in range(KSUB):
                            nc.tensor.matmul(
                                psum1[mc][:],
                                b_sb[g][:, ks * 512 + mc * 128:
                                        ks * 512 + (mc + 1) * 128],
                                xg[:, ks * 512:(ks + 1) * 512],
                                start=False,
                                stop=(ks == KSUB - 1),
                            )
                        nc.vector.tensor_copy(tT[b][mc][:], psum1[mc][:])

            def stage1_psum(b):
                return [
                    ps1p.tile([128, BROWS], F32, tag="ps1", name=f"ps1_{b}_{i}")
                    for i in range(RC)
                ]

            # ot tiles cover [128 rows, 2048 d-cols] (half the d range)
            def s2_psums(b, dcg, rc2s):
                return {
                    rc2: ps2p.tile([128, 512], F32, tag="ps2",
                                   name=f"ps2_{b}_{dcg}_{rc2}")
                    for rc2 in rc2s
                }

            def s2_mms(b, dcg, rc2, psum):
                for k in range(RC):
                    nc.tensor.matmul(
                        psum[:],
                        tT[b][k][:, rc2 * 128:(rc2 + 1) * 128],
                        at_sb[dcg][:, k * 512:(k + 1) * 512],
                        start=(k == 0),
                        stop=(k == RC - 1),
                    )

            # ---- stage1 block 0 ----
            ps_a = stage1_psum(0)
            for g in range(KB):
                stage1_group(0, g, ps_a)

            # ---- interleave: s1(1) group then s2(0) dcg-block ----
            ps_b = stage1_psum(1)
            ot0 = {}
            store_ring = [nc.gpsimd, nc.scalar, nc.gpsimd, nc.scalar]
            for dcg in range(DCG):
                if dcg % 2 == 0:
                    stage1_group(1, dcg, ps_b)
                    stage1_group(1, dcg + 1, ps_b)
                half, col = dcg // 4, dcg % 4
                if col == 0:
                    for rc2 in range(MB2):
                        ot0[(half, rc2)] = op.tile(
                            [128, 2048], BF16, tag="ot",
                            name=f"ot0_{half}_{rc2}",
                        )
                psums = s2_psums(0, dcg, range(MB2))
                for rc2 in range(MB2):
                    s2_mms(0, dcg, rc2, psums[rc2])
                    nc.vector.tensor_add(
                        ot0[(half, rc2)][:, col * 512:(col + 1) * 512],
                        psums[rc2][:],
                        bias_bc[:, dcg * 512:(dcg + 1) * 512],
                    )
                    if col == 3:
                        store_ring[rc2].dma_start(
                            out_d[rc2 * 128:rc2 * 128 + 128,
                                  half * 2048:(half + 1) * 2048],
                            ot0[(half, rc2)][:],
                        )

            # ---- stage2 block 1: rc2-major; last row-chunk stores fine ----
            fine_rings = [nc.gpsimd, nc.scalar, nc.sync, nc.gpsimd]
            for rc2 in range(MB2):
                last_rc2 = rc2 == MB2 - 1
                ot = {
                    half: op.tile([128, 2048], BF16, tag="ot",
                                  name=f"ot1_{rc2}_{half}")
                    for half in range(2)
                }
                for dcg in range(DCG):
                    half, col = dcg // 4, dcg % 4
                    psum = s2_psums(1, dcg, [rc2])[rc2]
                    s2_mms(1, dcg, rc2, psum)
                    row0 = BROWS + rc2 * 128
                    if last_rc2 and dcg == DCG - 1:
                        # very last tile: split add+store across both idle
                        # HWDGE rings to shorten the critical tail
                        for hf, ring in ((0, nc.sync), (1, nc.scalar)):
                            c0 = col * 512 + hf * 256
                            nc.vector.tensor_add(
                                ot[half][:, c0:c0 + 256],
                                psum[:, hf * 256:(hf + 1) * 256],
                                bias_bc[:, dcg * 512 + hf * 256:
                                        dcg * 512 + (hf + 1) * 256],
                            )
                            ring.dma_start(
                                out_d[row0:row0 + 128,
                                      dcg * 512 + hf * 256:
                                      dcg * 512 + (hf + 1) * 256],
                                ot[half][:, c0:c0 + 256],
                            )
                        continue
                    nc.vector.tensor_add(
                        ot[half][:, col * 512:(col + 1) * 512],
                        psum[:],
                        bias_bc[:, dcg * 512:(dcg + 1) * 512],
                    )
                    if last_rc2:
                        fine_rings[dcg % 4].dma_start(
                            out_d[row0:row0 + 128,
                                  dcg * 512:(dcg + 1) * 512],
                            ot[half][:, col * 512:(col + 1) * 512],
                        )
                    elif col == 3:
                        store_ring[rc2].dma_start(
                            out_d[row0:row0 + 128,
                                  half * 2048:(half + 1) * 2048],
                            ot[half][:],
                        )

    nc.compile()
    return nc


def _get_nc():
    if "nc" not in _compiled:
        _compiled["nc"] = _build()
    return _compiled["nc"]


def _prep_shared(A, B, bias):
    bf = ml_dtypes.bfloat16
    # b[g, p, ks*512+r] = B[g*512+ks*128+p, r]
    bh = np.ascontiguousarray(
        B.astype(bf).reshape(KB, KSUB, 128, RANK).transpose(0, 2, 1, 3)
    ).reshape(KB * 128, KSUB * RANK)
    # at[dcg, p, k*512+d] = A[dcg*512+d, k*128+p]
    ah = np.ascontiguousarray(
        A.astype(bf).reshape(DCG, 512, RC, 128).transpose(0, 3, 2, 1)
    ).reshape(DCG * 128, RC * 512)
    return bh, ah, np.ascontiguousarray(bias.astype(np.float32))


def _prep_x(xs):
    # x[b*8+g, p, ks*512+m] = xs[b*512+m, g*512+ks*128+p]
    bf = ml_dtypes.bfloat16
    xt = xs.astype(bf).reshape(NBLK, BROWS, KB, KSUB, 128)
    return np.ascontiguousarray(xt.transpose(0, 2, 4, 3, 1)).reshape(
        NBLK * KB * 128, KSUB * BROWS
    )


def run(inputs, trace=False, trace_kwargs=None):
    """Shard, execute on 8 cores, gather. Returns (output, BassKernelResults)."""
    x = np.asarray(inputs["x"], dtype=np.float32)
    A = np.asarray(inputs["A"], dtype=np.float32)
    B = np.asarray(inputs["B"], dtype=np.float32)
    bias = np.asarray(inputs["bias"], dtype=np.float32)

    x_flat = x.reshape(ROWS_TOTAL, D_IN)
    bh, ah, bias_f = _prep_shared(A, B, bias)
    in_maps = []
    for i in range(N_CORES):
        in_maps.append({
            "x": _prep_x(x_flat[i * ROWS:(i + 1) * ROWS]),
            "b": bh,
            "at": ah,
            "bias": bias_f,
        })

    nc = _get_nc()
    kwargs = {}
    if trace:
        kwargs["trace"] = True
        kwargs["trace_kwargs"] = trace_kwargs or {}
    res = None
    for attempt in range(3):
        try:
            res = run_bass_kernel_spmd(
                nc, in_maps, core_ids=list(range(N_CORES)), **kwargs
            )
        except Exception:
            # transient device/runtime hiccup; retry
            if attempt == 2:
                raise
            continue
        out = np.concatenate(
            [res.results[i]["out"].astype(np.float32) for i in range(N_CORES)],
            axis=0,
        )
        if np.isfinite(out).all():
            return out.reshape(BATCH, SEQ, D_OUT), res
    return out.reshape(BATCH, SEQ, D_OUT), res


def kernel(**inputs) -> np.ndarray:
    out, _ = run(inputs)
    return out


# revision 8
# speedup vs baseline: 1.0117x; 1.0117x over previous
"""TRN2 Bass kernel for CompressedLinearLayer: out = x @ (A @ B.T).T + bias.

Computed low-rank: t = x @ B  (rank 512), out = t @ A.T + bias.
Sharding: data-parallel over the 8192 rows of x (1024 rows per core);
B, A.T, bias replicated. No collectives.

Host pre-tiles every input into the exact SBUF layout so each device
DMA is one contiguous 0.5MB transfer (4KB per-partition lines):
  x  [16, 128, 2048] bf16   (block*8+g, p, ks*512+m)
  b  [ 8, 128, 2048] bf16   (g, p, ks*512+r)
  at [ 8, 128, 2048] bf16   (dcg, p, k*512+d)
  bias [4096] f32
  out [1024, 4096] bf16     (host casts back to f32)

Ring split: x on sync HWDGE, b then at on scalar HWDGE, bias + most
output stores on the gpsimd ring -- descriptor issue runs in parallel
and b/x stream concurrently from t~7.5us.  A.T arrives as 8
just-in-time 0.5MB column-block tiles consumed dcg-major by stage 2.

Per core the 1024 rows are processed in 2 blocks of 512:
  stage1(b): tT[r, m] = sum_k B[k, r] * xT[k, m]   (rank on partitions)
  stage2(b): out[m, d] = sum_r tT[r, m] * AT[r, d] + bias[d]
Interleave [s1(1)g0, s2dcg0, s1(1)g1, s2dcg1, ...] keeps the PE fed
across the block transition while tT evacuations drain on the DVE.
Accumulation is fp32 in PSUM; bias is added during PSUM evacuation on
the vector engine; outputs store as bf16 (final tiles fine-grained
across rings to shrink the tail).
"""
import numpy as np
import ml_dtypes

import concourse.bacc as bacc
import concourse.mybir as mybir
import concourse.tile as tile
from concourse.bass_utils import run_bass_kernel_spmd

N_CORES = 8
BATCH, SEQ = 4, 2048
D_IN, D_OUT, RANK = 4096, 4096, 512
ROWS_TOTAL = BATCH * SEQ           # 8192
ROWS = ROWS_TOTAL // N_CORES       # 1024 rows per core

F32 = mybir.dt.float32
BF16 = mybir.dt.bfloat16

KC = D_IN // 128     # 32 contraction chunks, stage 1
KSUB = 4             # k-chunks packed per DMA (0.5MB bf16 transfers)
KB = KC // KSUB      # 8 packed k-groups
RC = RANK // 128     # 4 rank chunks
NBLK = 2             # row blocks per core
BROWS = ROWS // NBLK                 # 512 rows per block
MB2 = BROWS // 128   # 4 row chunks of 128 per block (stage-2 out partitions)
DCG = D_OUT // 512   # 8 d_out column groups of 512 (stage-2 at tiles)

_compiled = {}


def _build():
    nc = bacc.Bacc("TRN2", target_bir_lowering=False, debug=False)

    x_d = nc.declare_dram_parameter("x", [NBLK * KB * 128, KSUB * BROWS], BF16,
                                    isOutput=False)
    b_d = nc.declare_dram_parameter("b", [KB * 128, KSUB * RANK], BF16,
                                    isOutput=False)
    at_d = nc.declare_dram_parameter("at", [DCG * 128, RC * 512], BF16,
                                     isOutput=False)
    bias_d = nc.declare_dram_parameter("bias", [D_OUT], F32, isOutput=False)
    out_d = nc.declare_dram_parameter("out", [ROWS, D_OUT], BF16, isOutput=True)

    with tile.TileContext(nc) as tc:
        with (
            tc.tile_pool(name="wb", bufs=1) as wb,
            tc.tile_pool(name="op", bufs=4) as op,
            tc.tile_pool(name="ps1", bufs=4, space="PSUM") as ps1p,
            tc.tile_pool(name="ps2", bufs=4, space="PSUM") as ps2p,
        ):
            bias_bc = wb.tile([128, D_OUT], F32, tag="bias_bc")

            # B resident: 8 tiles [128, 2048] bf16 (ks*512+r)
            b_sb = [
                wb.tile([128, KSUB * RANK], BF16, tag=f"b{g}", name=f"b{g}")
                for g in range(KB)
            ]
            # A.T resident: 8 column-group tiles [128, 2048] bf16 (k*512+d)
            at_sb = [
                wb.tile([128, RC * 512], BF16, tag=f"at{c}", name=f"at{c}")
                for c in range(DCG)
            ]
            # x: every group gets its own buffer (x0 + x1 fully resident)
            x_sb = [
                wb.tile([128, KSUB * BROWS], BF16, tag=f"x{i}", name=f"x{i}")
                for i in range(NBLK * KB)
            ]
            # tT per block: 4 tiles [128, 512] bf16 each
            tT = [
                [
                    wb.tile([128, BROWS], BF16, tag=f"tT{b}_{r}",
                            name=f"tT{b}_{r}")
                    for r in range(RC)
                ]
                for b in range(NBLK)
            ]

            # ---- critical first chunks (b g0ks0 + x g0ks0) on the sync
            # ring, which cold-starts ~1.5us earlier than scalar ----
            nc.sync.dma_start(b_sb[0][:, 0:512], b_d[0:128, 0:512])
            nc.sync.dma_start(x_sb[0][:, 0:512], x_d[0:128, 0:512])
            nc.sync.dma_start(x_sb[0][:, 512:2048], x_d[0:128, 512:2048])
            # remaining b on scalar
            nc.scalar.dma_start(b_sb[0][:, 512:2048], b_d[0:128, 512:2048])
            for g in range(1, KB):
                nc.scalar.dma_start(b_sb[g][:], b_d[g * 128:(g + 1) * 128, :])
            # rest of x on sync (3 active rings fragment DMA bandwidth, so
            # bulk transfers stay on the two HWDGE rings)
            for gi in range(1, NBLK * KB):
                nc.sync.dma_start(x_sb[gi][:], x_d[gi * 128:(gi + 1) * 128, :])
            # bias on the otherwise-idle gpsimd ring
            nc.gpsimd.dma_start(bias_bc[0:1, :], bias_d[None, :])
            nc.gpsimd.partition_broadcast(bias_bc[:], bias_bc[0:1, :])

            # ---- at on scalar ring, queued behind b ----
            for c in range(DCG):
                nc.scalar.dma_start(at_sb[c][:], at_d[c * 128:(c + 1) * 128, :])

            def stage1_group(b, g, psum1):
                xg = x_sb[b * KB + g]
                last = g == KB - 1
                if not last:
                    for ks in range(KSUB):
                        k = g * KSUB + ks
                        for mc in range(RC):
                            nc.tensor.matmul(
                                psum1[mc][:],
                                b_sb[g][:, ks * 512 + mc * 128:
                                        ks * 512 + (mc + 1) * 128],
                                xg[:, ks * 512:(ks + 1) * 512],
                                start=(k == 0),
                                stop=False,
                            )
                else:
                    # invert loops so each psum finishes (and can evacuate to
                    # tT on the DVE) while the PE continues with the next mc
                    for mc in range(RC):
                        for ks in range(KSUB):
                            nc.tensor.matmul(
                                psum1[mc][:],
                                b_sb[g][:, ks * 512 + mc * 128:
                                        ks * 512 + (mc + 1) * 128],
                                xg[:, ks * 512:(ks + 1) * 512],
                                start=False,
                                stop=(ks == KSUB - 1),
                            )
                        nc.vector.tensor_copy(tT[b][mc][:], psum1[mc][:])

            def stage1_psum(b):
                return [
                    ps1p.tile([128, BROWS], F32, tag="ps1", name=f"ps1_{b}_{i}")
                    for i in range(RC)
                ]

            # ot tiles cover [128 rows, 2048 d-cols] (half the d range)
            def s2_psums(b, dcg, rc2s):
                return {
                    rc2: ps2p.tile([128, 512], F32, tag="ps2",
                                   name=f"ps2_{b}_{dcg}_{rc2}")
                    for rc2 in rc2s
                }

            def s2_mms(b, dcg, rc2, psum):
                for k in range(RC):
                    nc.tensor.matmul(
                        psum[:],
                        tT[b][k][:, rc2 * 128:(rc2 + 1) * 128],
                        at_sb[dcg][:, k * 512:(k + 1) * 512],
                        start=(k == 0),
                        stop=(k == RC - 1),
                    )

            # ---- stage1 block 0 ----
            ps_a = stage1_psum(0)
            for g in range(KB):
                stage1_group(0, g, ps_a)

            # ---- interleave: s1(1) group then s2(0) dcg-block ----
            ps_b = stage1_psum(1)
            ot0 = {}
            store_ring = [nc.gpsimd, nc.scalar, nc.gpsimd, nc.scalar]
            for dcg in range(DCG):
                if dcg % 2 == 0:
                    stage1_group(1, dcg, ps_b)
                    stage1_group(1, dcg + 1, ps_b)
                half, col = dcg // 4, dcg % 4
                if col == 0:
                    for rc2 in range(MB2):
                        ot0[(half, rc2)] = op.tile(
                            [128, 2048], BF16, tag="ot",
                            name=f"ot0_{half}_{rc2}",
                        )
                psums = s2_psums(0, dcg, range(MB2))
                for rc2 in range(MB2):
                    s2_mms(0, dcg, rc2, psums[rc2])
                    nc.vector.tensor_add(
                        ot0[(half, rc2)][:, col * 512:(col + 1) * 512],
                        psums[rc2][:],
                        bias_bc[:, dcg * 512:(dcg + 1) * 512],
                    )
                    if col == 3:
                        store_ring[rc2].dma_start(
                            out_d[rc2 * 128:rc2 * 128 + 128,
                                  half * 2048:(half + 1) * 2048],
                            ot0[(half, rc2)][:],
                        )

            # ---- stage2 block 1: rc2-major; last row-chunk stores fine ----
            fine_rings = [nc.gpsimd, nc.scalar, nc.sync, nc.gpsimd]
            for rc2 in range(MB2):
                last_rc2 = rc2 == MB2 - 1
                ot = {
                    half: op.tile([128, 2048], BF16, tag="ot",
                                  name=f"ot1_{rc2}_{half}")
                    for half in range(2)
                }
                for dcg in range(DCG):
                    half, col = dcg // 4, dcg % 4
                    psum = s2_psums(1, dcg, [rc2])[rc2]
                    s2_mms(1, dcg, rc2, psum)
                    row0 = BROWS + rc2 * 128
                    if last_rc2 and dcg == DCG - 1:
                        # very last tile: split add+store across both idle
                        # HWDGE rings to shorten the critical tail
                        for hf, ring in ((0, nc.sync), (1, nc.scalar)):
                            c0 = col * 512 + hf * 256
                            nc.vector.tensor_add(
                                ot[half][:, c0:c0 + 256],
                                psum[:, hf * 256:(hf + 1) * 256],
                                bias_bc[:, dcg * 512 + hf * 256:
                                        dcg * 512 + (hf + 1) * 256],
                            )
                            ring.dma_start(
                                out_d[row0:row0 + 128,
                                      dcg * 512 + hf * 256:
                                      dcg * 512 + (hf + 1) * 256],
                                ot[half][:, c0:c0 + 256],
                            )
                        continue
                    nc.vector.tensor_add(
                        ot[half][:, col * 512:(col + 1) * 512],
                        psum[:],
                        bias_bc[:, dcg * 512:(dcg + 1) * 512],
                    )
                    if last_rc2:
                        fine_rings[dcg % 4].dma_start(
                            out_d[row0:row0 + 128,
                                  dcg * 512:(dcg + 1) * 512],
                            ot[half][:, col * 512:(col + 1) * 512],
                        )
                    elif col == 3:
                        store_ring[rc2].dma_start(
                            out_d[row0:row0 + 128,
                                  half * 2048:(half + 1) * 2048],
                            ot[half][:],
                        )

    nc.compile()
    return nc


def _get_nc():
    if "nc" not in _compiled:
        _compiled["nc"] = _build()
    return _compiled["nc"]


def _prep_shared(A, B, bias):
    bf = ml_dtypes.bfloat16
    # b[g, p, ks*512+r] = B[g*512+ks*128+p, r]
    bh = np.ascontiguousarray(
        B.astype(bf).reshape(KB, KSUB, 128, RANK).transpose(0, 2, 1, 3)
    ).reshape(KB * 128, KSUB * RANK)
    # at[dcg, p, k*512+d] = A[dcg*512+d, k*128+p]
    ah = np.ascontiguousarray(
        A.astype(bf).reshape(DCG, 512, RC, 128).transpose(0, 3, 2, 1)
    ).reshape(DCG * 128, RC * 512)
    return bh, ah, np.ascontiguousarray(bias.astype(np.float32))


def _prep_x(xs):
    # x[b*8+g, p, ks*512+m] = xs[b*512+m, g*512+ks*128+p]
    bf = ml_dtypes.bfloat16
    xt = xs.astype(bf).reshape(NBLK, BROWS, KB, KSUB, 128)
    return np.ascontiguousarray(xt.transpose(0, 2, 4, 3, 1)).reshape(
        NBLK * KB * 128, KSUB * BROWS
    )


def run(inputs, trace=False, trace_kwargs=None):
    """Shard, execute on 8 cores, gather. Returns (output, BassKernelResults)."""
    x = np.asarray(inputs["x"], dtype=np.float32)
    A = np.asarray(inputs["A"], dtype=np.float32)
    B = np.asarray(inputs["B"], dtype=np.float32)
    bias = np.asarray(inputs["bias"], dtype=np.float32)

    x_flat = x.reshape(ROWS_TOTAL, D_IN)
    bh, ah, bias_f = _prep_shared(A, B, bias)
    in_maps = []
    for i in range(N_CORES):
        in_maps.append({
            "x": _prep_x(x_flat[i * ROWS:(i + 1) * ROWS]),
            "b": bh,
            "at": ah,
            "bias": bias_f,
        })

    nc = _get_nc()
    kwargs = {}
    if trace:
        kwargs["trace"] = True
        kwargs["trace_kwargs"] = trace_kwargs or {}
    res = None
    for attempt in range(3):
        try:
            res = run_bass_kernel_spmd(
                nc, in_maps, core_ids=list(range(N_CORES)), **kwargs
            )
        except Exception:
            # transient device/runtime hiccup; retry
            if attempt == 2:
                raise
            continue
        out = np.concatenate(
            [res.results[i]["out"].astype(np.float32) for i in range(N_CORES)],
            axis=0,
        )
        if np.isfinite(out).all():
            return out.reshape(BATCH, SEQ, D_OUT), res
    return out.reshape(BATCH, SEQ, D_OUT), res


def kernel(**inputs) -> np.ndarray:
    out, _ = run(inputs)
    return out
